# revision 27
# baseline (speedup 1.0000x reference)
"""LIF Conv RNN Trainium2 kernel (8 NeuronCores, data-parallel over batch).

Reference semantics (per timestep t):
    i_t = conv3x3(x_t, Wx) + bx + conv3x3(h, Wh) + bh        (SAME padding)
    u   = DECAY * u * (1 - h) + i_t
    h   = (u > THRESH)
Output: h for every t -> [B, Ch, H, W, T].

Implementation notes:
  - 8-way data parallel over batch (2 batches per core), weights replicated.
  - Convs as 9 shifted-AP matmuls per pass accumulating in PSUM; each batch
    accumulates into its own [64, 512] PSUM bank (fp32r matmuls require
    partition-base-0 src/dst on this compiler).
  - float32r (e8m11) hi/lo split arithmetic: ~fp32-accuracy convolution at
    1 cycle/row on the PE (vs 4 for fp32): x = xh + xl exactly (Dekker
    split in e8m11); h is exact in e8m11 (binary). Three 9-tap matmul
    chains per (batch, row-half):
        S1=[xh;xl] x W1=[wxh;wxh]   -> conv(x, wxh)       (full-x precision)
        S2=[h;xh]  x WE=[whl;0]     -> conv(h, whl)
        S2=[h;xh]  x W2=[whh;wxl]   -> conv(h,whh)+conv(xh,wxl)
    (all five split products; dropping the whl chain was measured to
    destroy accuracy — rel err 0.28 — via recurrent spike cascades)
  - u recurrence in exact fp32 on the DVE:
        u = M_prev*DECAY + psum (+bias);  M = (u<=T)*u;  h = (u>T)
    with h written straight into the next step's padded S2 frame (is_gt
    with f32r output), which the y-output DMA also reads.
  - built on bacc.Bacc + nc.compile() so multi-wait sync legalization
    (EventSemaphore splitting) runs; the TRN2 ISA allows one wait/inst.
"""
import os
import numpy as np

import concourse.bass as bass
import concourse.bacc as bacc
import concourse.tile as tile

# this environment's axon build lacks the NTFF profile hook module that
# run_bass_kernel_spmd(trace=True) imports; stub it so tracing degrades
# to a plain run instead of crashing
import sys as _sys, types as _types
try:  # pragma: no cover
    import antenv.axon_hooks  # noqa: F401
except Exception:
    _m = _types.ModuleType("antenv.axon_hooks")
    _m.get_axon_ntff_profile_hook = lambda: None
    _sys.modules["antenv.axon_hooks"] = _m

PRED_NS = {}

# capture the Tile scheduler's simulated makespan (ns) at context exit —
# the only timing signal available in this environment (no NTFF profiling)
_orig_tc_exit = tile.TileContext.__exit__


def _tc_exit(self, *a):
    r = _orig_tc_exit(self, *a)
    try:
        PRED_NS["last"] = max(e[2] for e in self._perfetto_entries)
    except Exception:
        pass
    return r


tile.TileContext.__exit__ = _tc_exit


def _pred_span(tc, key):
    pass
import concourse.mybir as mybir
from concourse.bass_utils import run_bass_kernel_spmd
from concourse.tile_rust import add_dep_helper

DECAY = 0.3
THRESH = 0.5
B, Cin, H, W, T = 16, 64, 32, 32, 20
Ch = 64
NCORES = 8
BLOC = B // NCORES  # 2
P = 128
PW = H + 2
PWX = PW + 1  # +1 dead column for the DMA-absorber handshake
NPIX = H * W
AL = mybir.AluOpType
F32 = mybir.dt.float32
F32R = mybir.dt.float32r

SCHEME = os.environ.get("LIF_SCHEME", "rs")  # "rs" (f32r split) | "f32"
DROP_E = os.environ.get("LIF_DROP_E", "0") == "1"


def to_f32r(a):
    """Round fp32 -> e8m11 (float32r) RNE, keeping fp32 bit layout."""
    b = np.ascontiguousarray(a, dtype=np.float32).view(np.uint32)
    add = ((b >> 12) & 1) + 0x7FF
    return ((b + add) & 0xFFFFF000).astype(np.uint32).view(np.float32)


def _win(xt, tap, r):
    dy, dx = divmod(tap, 3)
    return xt[:, dy + 16 * r: dy + 16 * r + 16, dx: dx + W]


def _build_rs(has_bias):
    nc = bacc.Bacc()
    d_xS1 = nc.dram_tensor("xS1", [T, BLOC, P, PW, PWX], F32R, kind="ExternalInput")
    d_xS2 = nc.dram_tensor("xS2", [T, BLOC, Ch, PW, PWX], F32R, kind="ExternalInput")
    d_W1 = nc.dram_tensor("W1", [9, P, Ch], F32R, kind="ExternalInput")
    d_W2 = nc.dram_tensor("W2", [9, P, Ch], F32R, kind="ExternalInput")
    d_WE = nc.dram_tensor("WE", [9, P, Ch], F32R, kind="ExternalInput")
    d_bias = nc.dram_tensor("bias", [P, 1], F32, kind="ExternalInput")
    d_y = nc.dram_tensor("y", [T, BLOC, Ch, H, W], F32, kind="ExternalOutput")

    with tile.TileContext(nc) as tc:
        with (
            tc.tile_pool(name="const", bufs=1) as cp,
            tc.tile_pool(name="state", bufs=1) as st,
            tc.tile_pool(name="work", bufs=2) as wk,
            tc.tile_pool(name="ps", bufs=6, space="PSUM") as ps,
        ):
            tW1 = cp.tile([P, 9, Ch], F32R, tag="w1")
            tW2 = cp.tile([P, 9, Ch], F32R, tag="w2")
            tWE = cp.tile([P, 9, Ch], F32R, tag="we")
            nc.sync.dma_start(tW1[:], d_W1[:].rearrange("t p c -> p t c"))
            nc.sync.dma_start(tW2[:], d_W2[:].rearrange("t p c -> p t c"))
            nc.sync.dma_start(tWE[:], d_WE[:].rearrange("t p c -> p t c"))
            tb = cp.tile([P, 1], F32, tag="bias")
            nc.sync.dma_start(tb[:], d_bias[:])

            # S1 = [xh; xl]; S2 = [h(0:64); xh(64:128)] for BOTH batches —
            # fp32r matmuls require dst/src partition base 0, so each batch
            # accumulates into its own [64, 512] PSUM bank instead of
            # complementary partition halves.
            NSLOT = 3
            S1 = [[st.tile([P, PW, PWX], F32R, tag=f"s1_{b}{i}", name=f"s1_{b}{i}")
                   for i in range(NSLOT)] for b in range(2)]
            S2 = [[st.tile([P, PW, PWX], F32R, tag=f"s2_{b}{i}", name=f"s2_{b}{i}")
                   for i in range(NSLOT)] for b in range(2)]
            for b in range(2):
                for i in range(NSLOT):
                    nc.vector.memset(S2[b][i][0:Ch, :, :].bitcast(F32), 0.0)

            def dma_x(t, slot):
                for b in range(2):
                    nc.sync.dma_start(S1[b][slot][:], d_xS1[t, b])
                    nc.sync.dma_start(S2[b][slot][Ch:P, :, :], d_xS2[t, b])

            dma_x(0, 0)
            dma_x(1, 1)
            tMd = [None, None]
            for t in range(T):
                cur, nxt = t % NSLOT, (t + 1) % NSLOT
                accs = [[ps.tile([Ch, 512], F32, tag="acc", name="acc")
                         for _ in range(2)] for _ in range(2)]  # [b][r]
                # mm1: x-only products (h-independent), overlaps prev-step DVE
                for r in range(2):
                    for b in range(2):
                        for tap in range(9):
                            nc.tensor.matmul(
                                accs[b][r][:], tW1[:, tap, :],
                                _win(S1[b][cur], tap, r),
                                start=(tap == 0), stop=False,
                                skip_group_check=True)
                # E (h*whl; lhsT zero-padded to K=128) then mm2 ([h|xh])
                for r in range(2):
                    for b in range(2):
                        if not DROP_E:
                            for tap in range(9):
                                nc.tensor.matmul(
                                    accs[b][r][:], tWE[:, tap, :],
                                    _win(S2[b][cur], tap, r),
                                    start=False, stop=False,
                                    skip_group_check=True)
                        for tap in range(9):
                            nc.tensor.matmul(
                                accs[b][r][:], tW2[:, tap, :],
                                _win(S2[b][cur], tap, r),
                                start=False, stop=(tap == 8),
                                skip_group_check=True)
                if t + 2 < T:
                    dma_x(t + 2, (t + 2) % NSLOT)
                tus = []
                for b in range(2):
                    tu = wk.tile([Ch, NPIX], F32, tag=f"u{b}", name=f"u{b}")
                    for r in range(2):
                        if tMd[b] is None:
                            nc.vector.tensor_scalar(
                                tu[:, 512 * r: 512 * (r + 1)], accs[b][r][:],
                                0.0, None, AL.add)
                        else:
                            nc.vector.scalar_tensor_tensor(
                                tu[:, 512 * r: 512 * (r + 1)],
                                tMd[b][:, 512 * r: 512 * (r + 1)], DECAY,
                                accs[b][r][:], AL.mult, AL.add)
                    if has_bias:
                        nc.vector.tensor_scalar(tu[:], tu[:], tb[0:Ch, :],
                                                None, AL.add)
                    tus.append(tu)
                for b in range(2):
                    tMd_new = wk.tile([Ch, NPIX], F32, tag=f"md{b}", name=f"md{b}")
                    nc.vector.scalar_tensor_tensor(
                        tMd_new[:], tus[b][:], THRESH, tus[b][:],
                        AL.is_le, AL.mult)
                    tMd[b] = tMd_new
                    nc.vector.tensor_scalar(
                        S2[b][nxt][0:Ch, 1:33, 1:33],
                        tus[b][:].rearrange("c (h w) -> c h w", h=H),
                        THRESH, None, AL.is_gt)
                    nc.sync.dma_start(
                        d_y[t, b], S2[b][nxt][0:Ch, 1:33, 1:33].bitcast(F32))
        _pred_span(tc, "rs")
    nc.compile()
    return nc


def _build_f32(has_bias):
    """Plain fp32: stacked [x; h] K=128 conv, 4 cycles/row (fallback)."""
    nc = bacc.Bacc()
    d_x = nc.dram_tensor("xS1", [T, BLOC, Ch, PW, PWX], F32, kind="ExternalInput")
    d_Wb0 = nc.dram_tensor("W2b0", [9, P, Ch], F32, kind="ExternalInput")
    d_Wb1 = nc.dram_tensor("W2b1", [9, P, Ch], F32, kind="ExternalInput")
    d_bias = nc.dram_tensor("bias", [P, 1], F32, kind="ExternalInput")
    d_y = nc.dram_tensor("y", [T, BLOC, Ch, H, W], F32, kind="ExternalOutput")

    with tile.TileContext(nc) as tc:
        with (
            tc.tile_pool(name="const", bufs=1) as cp,
            tc.tile_pool(name="state", bufs=1) as st,
            tc.tile_pool(name="work", bufs=2) as wk,
            tc.tile_pool(name="ps", bufs=6, space="PSUM") as ps,
            tc.tile_pool(name="pwarm", bufs=1, space="PSUM") as pw,
        ):
            tWb0 = cp.tile([P, 9, Ch], F32, tag="wb0")
            tWb1 = cp.tile([P, 9, Ch], F32, tag="wb1")
            nc.sync.dma_start(tWb0[:], d_Wb0[:].rearrange("t p c -> p t c"))
            nc.sync.dma_start(tWb1[:], d_Wb1[:].rearrange("t p c -> p t c"))
            tb = cp.tile([P, 1], F32, tag="bias")
            nc.sync.dma_start(tb[:], d_bias[:])

            # b0: [x(0:64); h(64:128)], b1: [h(0:64); x(64:128)]
            Sb0 = [st.tile([P, PW, PWX], F32, tag=f"sb0_{i}", name=f"sb0_{i}") for i in range(2)]
            Sb1 = [st.tile([P, PW, PWX], F32, tag=f"sb1_{i}", name=f"sb1_{i}") for i in range(2)]
            for i in range(2):
                nc.vector.memset(Sb0[i][Ch:P, :, :], 0.0)
                nc.vector.memset(Sb1[i][0:Ch, :, :], 0.0)

            scratch = pw.tile([1, 16], F32, tag="warm")
            zcol = cp.tile([P, 1], F32, tag="zcol")
            nc.vector.memset(zcol[:], 0.0)
            warm = None
            for wt in (tWb0, tWb1):
                warm = nc.tensor.matmul(scratch[0:1, 0:1], wt[:, 0, 0:1],
                                        wt[:, 0, 0:1], start=True, stop=True)

            def dma_x(t, slot):
                nc.sync.dma_start(Sb0[slot][0:Ch, :, :], d_x[t, 0])
                nc.sync.dma_start(Sb1[slot][Ch:P, :, :], d_x[t, 1])

            dma_x(0, 0)
            tMd = None
            for t in range(T):
                cur, nxt = t % 2, (t + 1) % 2
                accs = [ps.tile([P, 512], F32, tag="acc", name="acc") for _ in range(2)]
                # h warmers first: tiny scratch matmuls reading an interior
                # column of each h frame — absorb the DVE h-write ticks on
                # the PE (ahead of the zero warmers so the latters' PSUM-slot
                # DVE requirements are already observed)
                wh0 = nc.tensor.matmul(
                    scratch[0:1, 0:1], Sb0[cur][Ch:P, 1, 1:2],
                    Sb0[cur][Ch:P, 1, 1:2], start=True, stop=True,
                    tile_position=(64, 0), skip_group_check=True)
                if t == 0:
                    add_dep_helper(wh0.ins, warm.ins, sync=False,
                                   reason="order after weight warmers")
                wh1 = nc.tensor.matmul(
                    scratch[0:1, 0:1], Sb1[cur][0:Ch, 1, 1:2],
                    Sb1[cur][0:Ch, 1, 1:2], start=True, stop=True,
                    tile_position=(0, 0), skip_group_check=True)
                add_dep_helper(wh1.ins, wh0.ins, sync=False, reason="order")
                warms = []
                for r in range(2):
                    # zero warmer: absorbs the PSUM-slot PE wait + clears the
                    # bank (0*0 product) so real matmuls run start=False
                    wz = nc.tensor.matmul(
                        accs[r][Ch:Ch + 1, 0:1], zcol[:], zcol[:],
                        start=True, stop=False, tile_position=(0, 64),
                        skip_group_check=True)
                    add_dep_helper(wz.ins, wh1.ins, sync=False,
                                   reason="order after h warmers")
                    warms.append(wz)
                for r in range(2):
                    acc = accs[r]
                    for tap in range(9):
                        m0 = nc.tensor.matmul(
                            acc[Ch:P, :], tWb0[:, tap, :], _win(Sb0[cur], tap, r),
                            start=False, stop=(tap == 8), tile_position=(0, 64),
                            skip_group_check=True)
                        m1 = nc.tensor.matmul(
                            acc[0:Ch, :], tWb1[:, tap, :], _win(Sb1[cur], tap, r),
                            start=False, stop=(tap == 8), tile_position=(0, 0),
                            skip_group_check=True)
                        if tap == 0:
                            add_dep_helper(m0.ins, wh1.ins, sync=False,
                                           reason="after h warmers")
                            add_dep_helper(m1.ins, wh1.ins, sync=False,
                                           reason="after h warmers")
                            add_dep_helper(m0.ins, warms[r].ins, sync=False,
                                           reason="after bank clear")
                            add_dep_helper(m1.ins, warms[r].ins, sync=False,
                                           reason="after bank clear")
                if t + 1 < T:
                    dma_x(t + 1, nxt)
                tu = wk.tile([P, NPIX], F32, tag="u")
                for r in range(2):
                    if tMd is None:
                        nc.vector.tensor_scalar(
                            tu[:, 512 * r: 512 * (r + 1)], accs[r][:],
                            0.0, None, AL.add)
                    else:
                        nc.vector.scalar_tensor_tensor(
                            tu[:, 512 * r: 512 * (r + 1)],
                            tMd[:, 512 * r: 512 * (r + 1)], DECAY, accs[r][:],
                            AL.mult, AL.add)
                if has_bias:
                    nc.vector.tensor_scalar(tu[:], tu[:], tb[:], None, AL.add)
                tMd_new = wk.tile([P, NPIX], F32, tag="md")
                nc.vector.scalar_tensor_tensor(
                    tMd_new[:], tu[:], THRESH, tu[:], AL.is_le, AL.mult)
                nc.vector.tensor_scalar(
                    Sb0[nxt][Ch:P, 1:33, 1:33],
                    tu[Ch:P, :].rearrange("c (h w) -> c h w", h=H),
                    THRESH, None, AL.is_gt)
                nc.vector.tensor_scalar(
                    Sb1[nxt][0:Ch, 1:33, 1:33],
                    tu[0:Ch, :].rearrange("c (h w) -> c h w", h=H),
                    THRESH, None, AL.is_gt)
                nc.sync.dma_start(d_y[t, 0], Sb0[nxt][Ch:P, 1:33, 1:33])
                nc.sync.dma_start(d_y[t, 1], Sb1[nxt][0:Ch, 1:33, 1:33])
                tMd = tMd_new
        _pred_span(tc, "f32")
    nc.compile()
    return nc


_cache = {}


def _get_nc(has_bias):
    key = (SCHEME, DROP_E, has_bias)
    if key not in _cache:
        _cache[key] = _build_rs(has_bias) if SCHEME == "rs" else _build_f32(has_bias)
    return _cache[key]


def _pad_frames(a):
    """[..., H, W] -> [..., PW, PWX] zero-padded frames (+dead col)."""
    out = np.zeros(a.shape[:-2] + (PW, PWX), dtype=a.dtype)
    out[..., 1:33, 1:33] = a
    return out


def kernel(data, Wx, bx, Wh, bh, _trace=False):
    data = np.asarray(data, dtype=np.float32)
    Wx = np.asarray(Wx, dtype=np.float32)
    Wh = np.asarray(Wh, dtype=np.float32)
    bx = np.asarray(bx, dtype=np.float32)
    bh = np.asarray(bh, dtype=np.float32)

    # lhsT tap layout: [tap(ky*3+kx), cin, cout]
    WxT = np.ascontiguousarray(np.transpose(Wx, (2, 3, 1, 0)).reshape(9, Cin, Ch))
    WhT = np.ascontiguousarray(np.transpose(Wh, (2, 3, 1, 0)).reshape(9, Ch, Ch))
    bsum = (bx + bh).astype(np.float32)
    has_bias = bool(np.any(bsum))
    bias_v = np.concatenate([bsum, bsum])[:, None]

    nc = _get_nc(has_bias)

    in_maps = []
    if SCHEME == "rs":
        wxh = to_f32r(WxT); wxl = to_f32r(WxT - wxh)
        whh = to_f32r(WhT); whl = to_f32r(WhT - whh)
        W1 = np.concatenate([wxh, wxh], axis=1)          # S1 = [xh; xl]
        W2 = np.concatenate([whh, wxl], axis=1)          # S2 = [h; xh]
        WE = np.concatenate([whl, np.zeros_like(whl)], axis=1)
        for c in range(NCORES):
            xc = np.moveaxis(data[BLOC * c: BLOC * (c + 1)], -1, 0)  # [T,2,64,32,32]
            xh = to_f32r(xc)
            xl = to_f32r(xc - xh)
            xS1 = _pad_frames(np.concatenate([xh, xl], axis=2))
            xS2 = _pad_frames(xh)
            in_maps.append({"xS1": xS1, "xS2": xS2, "W1": W1, "W2": W2,
                            "WE": WE, "bias": bias_v})
    else:
        Wb0 = np.concatenate([WxT, WhT], axis=1)  # b0: [x; h]
        Wb1 = np.concatenate([WhT, WxT], axis=1)  # b1: [h; x]
        for c in range(NCORES):
            xc = _pad_frames(np.moveaxis(data[BLOC * c: BLOC * (c + 1)], -1, 0))
            in_maps.append({"xS1": xc, "W2b0": Wb0, "W2b1": Wb1, "bias": bias_v})

    res = run_bass_kernel_spmd(nc, in_maps, core_ids=list(range(NCORES)),
                               trace=_trace)
    out = np.empty((B, Ch, H, W, T), dtype=np.float32)
    for c in range(NCORES):
        y = res.results[c]["y"]  # [T, 2, 64, 32, 32]
        for j in range(BLOC):
            out[BLOC * c + j] = np.transpose(y[:, j], (1, 2, 3, 0))
    if _trace:
        return out, res
    return out


# revision 28
# speedup vs baseline: 1.0029x; 1.0029x over previous
"""LIF Conv RNN Trainium2 kernel (8 NeuronCores, data-parallel over batch).

Reference semantics (per timestep t):
    i_t = conv3x3(x_t, Wx) + bx + conv3x3(h, Wh) + bh        (SAME padding)
    u   = DECAY * u * (1 - h) + i_t
    h   = (u > THRESH)
Output: h for every t -> [B, Ch, H, W, T].

Implementation notes:
  - 8-way data parallel over batch (2 batches per core), weights replicated.
  - Convs as 9 shifted-AP matmuls per pass accumulating in PSUM; each batch
    accumulates into its own [64, 512] PSUM bank (fp32r matmuls require
    partition-base-0 src/dst on this compiler).
  - float32r (e8m11) hi/lo split arithmetic: ~fp32-accuracy convolution at
    1 cycle/row on the PE (vs 4 for fp32): x = xh + xl exactly (Dekker
    split in e8m11); h is exact in e8m11 (binary). Three 9-tap matmul
    chains per (batch, row-half):
        S1=[xh;xl] x W1=[wxh;wxh]   -> conv(x, wxh)       (full-x precision)
        S2=[h;xh]  x WE=[whl;0]     -> conv(h, whl)
        S2=[h;xh]  x W2=[whh;wxl]   -> conv(h,whh)+conv(xh,wxl)
    (all five split products; dropping the whl chain was measured to
    destroy accuracy — rel err 0.28 — via recurrent spike cascades)
  - u recurrence in exact fp32 on the DVE:
        u = M_prev*DECAY + psum (+bias);  M = (u<=T)*u;  h = (u>T)
    with h written straight into the next step's padded S2 frame (is_gt
    with f32r output), which the y-output DMA also reads.
  - built on bacc.Bacc + nc.compile() so multi-wait sync legalization
    (EventSemaphore splitting) runs; the TRN2 ISA allows one wait/inst.
"""
import os
import numpy as np

import concourse.bass as bass
import concourse.bacc as bacc
import concourse.tile as tile

# this environment's axon build lacks the NTFF profile hook module that
# run_bass_kernel_spmd(trace=True) imports; stub it so tracing degrades
# to a plain run instead of crashing
import sys as _sys, types as _types
try:  # pragma: no cover
    import antenv.axon_hooks  # noqa: F401
except Exception:
    _m = _types.ModuleType("antenv.axon_hooks")
    _m.get_axon_ntff_profile_hook = lambda: None
    _sys.modules["antenv.axon_hooks"] = _m

PRED_NS = {}

# capture the Tile scheduler's simulated makespan (ns) at context exit —
# the only timing signal available in this environment (no NTFF profiling)
_orig_tc_exit = tile.TileContext.__exit__


def _tc_exit(self, *a):
    r = _orig_tc_exit(self, *a)
    try:
        PRED_NS["last"] = max(e[2] for e in self._perfetto_entries)
    except Exception:
        pass
    return r


tile.TileContext.__exit__ = _tc_exit


def _pred_span(tc, key):
    pass
import concourse.mybir as mybir
from concourse.bass_utils import run_bass_kernel_spmd
from concourse.tile_rust import add_dep_helper

DECAY = 0.3
THRESH = 0.5
B, Cin, H, W, T = 16, 64, 32, 32, 20
Ch = 64
NCORES = 8
BLOC = B // NCORES  # 2
P = 128
PW = H + 2
PWX = PW + 1  # +1 dead column for the DMA-absorber handshake
NPIX = H * W
AL = mybir.AluOpType
F32 = mybir.dt.float32
F32R = mybir.dt.float32r

SCHEME = os.environ.get("LIF_SCHEME", "rs")  # "rs" (f32r split) | "f32"
DROP_E = os.environ.get("LIF_DROP_E", "0") == "1"


def to_f32r(a):
    """Round fp32 -> e8m11 (float32r) RNE, keeping fp32 bit layout."""
    b = np.ascontiguousarray(a, dtype=np.float32).view(np.uint32)
    add = ((b >> 12) & 1) + 0x7FF
    return ((b + add) & 0xFFFFF000).astype(np.uint32).view(np.float32)


def _win(xt, tap, r):
    dy, dx = divmod(tap, 3)
    return xt[:, dy + 16 * r: dy + 16 * r + 16, dx: dx + W]


def _build_rs(has_bias):
    nc = bacc.Bacc()
    d_xS1 = nc.dram_tensor("xS1", [T, BLOC, P, PW, PWX], F32R, kind="ExternalInput")
    d_xS2 = nc.dram_tensor("xS2", [T, BLOC, Ch, PW, PWX], F32R, kind="ExternalInput")
    d_W1 = nc.dram_tensor("W1", [9, P, Ch], F32R, kind="ExternalInput")
    d_W2 = nc.dram_tensor("W2", [9, P, Ch], F32R, kind="ExternalInput")
    d_WE = nc.dram_tensor("WE", [9, P, Ch], F32R, kind="ExternalInput")
    d_bias = nc.dram_tensor("bias", [P, 1], F32, kind="ExternalInput")
    d_y = nc.dram_tensor("y", [T, BLOC, Ch, H, W], F32, kind="ExternalOutput")

    with tile.TileContext(nc) as tc:
        with (
            tc.tile_pool(name="const", bufs=1) as cp,
            tc.tile_pool(name="state", bufs=1) as st,
            tc.tile_pool(name="work", bufs=2) as wk,
            tc.tile_pool(name="ps", bufs=6, space="PSUM") as ps,
        ):
            tW1 = cp.tile([P, 9, Ch], F32R, tag="w1")
            tW2 = cp.tile([P, 9, Ch], F32R, tag="w2")
            tWE = cp.tile([P, 9, Ch], F32R, tag="we")
            nc.sync.dma_start(tW1[:], d_W1[:].rearrange("t p c -> p t c"))
            nc.sync.dma_start(tW2[:], d_W2[:].rearrange("t p c -> p t c"))
            nc.sync.dma_start(tWE[:], d_WE[:].rearrange("t p c -> p t c"))
            tb = cp.tile([P, 1], F32, tag="bias")
            nc.sync.dma_start(tb[:], d_bias[:])

            # S1 = [xh; xl]; S2 = [h(0:64); xh(64:128)] for BOTH batches —
            # fp32r matmuls require dst/src partition base 0, so each batch
            # accumulates into its own [64, 512] PSUM bank instead of
            # complementary partition halves.
            NSLOT = 3
            S1 = [[st.tile([P, PW, PWX], F32R, tag=f"s1_{b}{i}", name=f"s1_{b}{i}")
                   for i in range(NSLOT)] for b in range(2)]
            S2 = [[st.tile([P, PW, PWX], F32R, tag=f"s2_{b}{i}", name=f"s2_{b}{i}")
                   for i in range(NSLOT)] for b in range(2)]
            for b in range(2):
                for i in range(NSLOT):
                    nc.vector.memset(S2[b][i][0:Ch, :, :].bitcast(F32), 0.0)

            def dma_x(t, slot):
                for b in range(2):
                    nc.sync.dma_start(S1[b][slot][:], d_xS1[t, b])
                    nc.sync.dma_start(S2[b][slot][Ch:P, :, :], d_xS2[t, b])

            dma_x(0, 0)
            dma_x(1, 1)
            tMd = [None, None]
            for t in range(T):
                cur, nxt = t % NSLOT, (t + 1) % NSLOT
                accs = [[ps.tile([Ch, 512], F32, tag="acc", name="acc")
                         for _ in range(2)] for _ in range(2)]  # [b][r]
                # mm1: x-only products (h-independent), overlaps prev-step DVE
                for r in range(2):
                    for b in range(2):
                        for tap in range(9):
                            nc.tensor.matmul(
                                accs[b][r][:], tW1[:, tap, :],
                                _win(S1[b][cur], tap, r),
                                start=(tap == 0), stop=False,
                                skip_group_check=True)
                # E (h*whl; lhsT zero-padded to K=128) then mm2 ([h|xh])
                for r in range(2):
                    for b in range(2):
                        if not DROP_E:
                            for tap in range(9):
                                nc.tensor.matmul(
                                    accs[b][r][:], tWE[:, tap, :],
                                    _win(S2[b][cur], tap, r),
                                    start=False, stop=False,
                                    skip_group_check=True)
                        for tap in range(9):
                            nc.tensor.matmul(
                                accs[b][r][:], tW2[:, tap, :],
                                _win(S2[b][cur], tap, r),
                                start=False, stop=(tap == 8),
                                skip_group_check=True)
                if t + 2 < T:
                    dma_x(t + 2, (t + 2) % NSLOT)
                # u and the spike-writes are the recurrent critical path:
                # emit them per (batch, row-half) so each half fires as soon
                # as its PSUM bank stops, and defer M (a full timestep of
                # slack before its next use) until after the h-writes.
                tus = []
                for b in range(2):
                    tu = wk.tile([Ch, NPIX], F32, tag=f"u{b}", name=f"u{b}")
                    tus.append(tu)
                for r in range(2):
                    for b in range(2):
                        tu = tus[b]
                        if tMd[b] is None:
                            nc.vector.tensor_scalar(
                                tu[:, 512 * r: 512 * (r + 1)], accs[b][r][:],
                                0.0, None, AL.add)
                        else:
                            nc.vector.scalar_tensor_tensor(
                                tu[:, 512 * r: 512 * (r + 1)],
                                tMd[b][:, 512 * r: 512 * (r + 1)], DECAY,
                                accs[b][r][:], AL.mult, AL.add)
                        if has_bias:
                            nc.vector.tensor_scalar(
                                tu[:, 512 * r: 512 * (r + 1)],
                                tu[:, 512 * r: 512 * (r + 1)],
                                tb[0:Ch, :], None, AL.add)
                        # spike-write for this half: rows 16r..16r+15 of the
                        # interior frame (rows 1+16r..16+16r of the padding)
                        nc.vector.tensor_scalar(
                            S2[b][nxt][0:Ch, 1 + 16 * r: 17 + 16 * r, 1:33],
                            tu[:, 512 * r: 512 * (r + 1)].rearrange(
                                "c (h w) -> c h w", h=16),
                            THRESH, None, AL.is_gt)
                for b in range(2):
                    tMd_new = wk.tile([Ch, NPIX], F32, tag=f"md{b}", name=f"md{b}")
                    nc.vector.scalar_tensor_tensor(
                        tMd_new[:], tus[b][:], THRESH, tus[b][:],
                        AL.is_le, AL.mult)
                    tMd[b] = tMd_new
                    nc.sync.dma_start(
                        d_y[t, b], S2[b][nxt][0:Ch, 1:33, 1:33].bitcast(F32))
        _pred_span(tc, "rs")
    nc.compile()
    return nc


def _build_f32(has_bias):
    """Plain fp32: stacked [x; h] K=128 conv, 4 cycles/row (fallback)."""
    nc = bacc.Bacc()
    d_x = nc.dram_tensor("xS1", [T, BLOC, Ch, PW, PWX], F32, kind="ExternalInput")
    d_Wb0 = nc.dram_tensor("W2b0", [9, P, Ch], F32, kind="ExternalInput")
    d_Wb1 = nc.dram_tensor("W2b1", [9, P, Ch], F32, kind="ExternalInput")
    d_bias = nc.dram_tensor("bias", [P, 1], F32, kind="ExternalInput")
    d_y = nc.dram_tensor("y", [T, BLOC, Ch, H, W], F32, kind="ExternalOutput")

    with tile.TileContext(nc) as tc:
        with (
            tc.tile_pool(name="const", bufs=1) as cp,
            tc.tile_pool(name="state", bufs=1) as st,
            tc.tile_pool(name="work", bufs=2) as wk,
            tc.tile_pool(name="ps", bufs=6, space="PSUM") as ps,
            tc.tile_pool(name="pwarm", bufs=1, space="PSUM") as pw,
        ):
            tWb0 = cp.tile([P, 9, Ch], F32, tag="wb0")
            tWb1 = cp.tile([P, 9, Ch], F32, tag="wb1")
            nc.sync.dma_start(tWb0[:], d_Wb0[:].rearrange("t p c -> p t c"))
            nc.sync.dma_start(tWb1[:], d_Wb1[:].rearrange("t p c -> p t c"))
            tb = cp.tile([P, 1], F32, tag="bias")
            nc.sync.dma_start(tb[:], d_bias[:])

            # b0: [x(0:64); h(64:128)], b1: [h(0:64); x(64:128)]
            Sb0 = [st.tile([P, PW, PWX], F32, tag=f"sb0_{i}", name=f"sb0_{i}") for i in range(2)]
            Sb1 = [st.tile([P, PW, PWX], F32, tag=f"sb1_{i}", name=f"sb1_{i}") for i in range(2)]
            for i in range(2):
                nc.vector.memset(Sb0[i][Ch:P, :, :], 0.0)
                nc.vector.memset(Sb1[i][0:Ch, :, :], 0.0)

            scratch = pw.tile([1, 16], F32, tag="warm")
            zcol = cp.tile([P, 1], F32, tag="zcol")
            nc.vector.memset(zcol[:], 0.0)
            warm = None
            for wt in (tWb0, tWb1):
                warm = nc.tensor.matmul(scratch[0:1, 0:1], wt[:, 0, 0:1],
                                        wt[:, 0, 0:1], start=True, stop=True)

            def dma_x(t, slot):
                nc.sync.dma_start(Sb0[slot][0:Ch, :, :], d_x[t, 0])
                nc.sync.dma_start(Sb1[slot][Ch:P, :, :], d_x[t, 1])

            dma_x(0, 0)
            tMd = None
            for t in range(T):
                cur, nxt = t % 2, (t + 1) % 2
                accs = [ps.tile([P, 512], F32, tag="acc", name="acc") for _ in range(2)]
                # h warmers first: tiny scratch matmuls reading an interior
                # column of each h frame — absorb the DVE h-write ticks on
                # the PE (ahead of the zero warmers so the latters' PSUM-slot
                # DVE requirements are already observed)
                wh0 = nc.tensor.matmul(
                    scratch[0:1, 0:1], Sb0[cur][Ch:P, 1, 1:2],
                    Sb0[cur][Ch:P, 1, 1:2], start=True, stop=True,
                    tile_position=(64, 0), skip_group_check=True)
                if t == 0:
                    add_dep_helper(wh0.ins, warm.ins, sync=False,
                                   reason="order after weight warmers")
                wh1 = nc.tensor.matmul(
                    scratch[0:1, 0:1], Sb1[cur][0:Ch, 1, 1:2],
                    Sb1[cur][0:Ch, 1, 1:2], start=True, stop=True,
                    tile_position=(0, 0), skip_group_check=True)
                add_dep_helper(wh1.ins, wh0.ins, sync=False, reason="order")
                warms = []
                for r in range(2):
                    # zero warmer: absorbs the PSUM-slot PE wait + clears the
                    # bank (0*0 product) so real matmuls run start=False
                    wz = nc.tensor.matmul(
                        accs[r][Ch:Ch + 1, 0:1], zcol[:], zcol[:],
                        start=True, stop=False, tile_position=(0, 64),
                        skip_group_check=True)
                    add_dep_helper(wz.ins, wh1.ins, sync=False,
                                   reason="order after h warmers")
                    warms.append(wz)
                for r in range(2):
                    acc = accs[r]
                    for tap in range(9):
                        m0 = nc.tensor.matmul(
                            acc[Ch:P, :], tWb0[:, tap, :], _win(Sb0[cur], tap, r),
                            start=False, stop=(tap == 8), tile_position=(0, 64),
                            skip_group_check=True)
                        m1 = nc.tensor.matmul(
                            acc[0:Ch, :], tWb1[:, tap, :], _win(Sb1[cur], tap, r),
                            start=False, stop=(tap == 8), tile_position=(0, 0),
                            skip_group_check=True)
                        if tap == 0:
                            add_dep_helper(m0.ins, wh1.ins, sync=False,
                                           reason="after h warmers")
                            add_dep_helper(m1.ins, wh1.ins, sync=False,
                                           reason="after h warmers")
                            add_dep_helper(m0.ins, warms[r].ins, sync=False,
                                           reason="after bank clear")
                            add_dep_helper(m1.ins, warms[r].ins, sync=False,
                                           reason="after bank clear")
                if t + 1 < T:
                    dma_x(t + 1, nxt)
                tu = wk.tile([P, NPIX], F32, tag="u")
                for r in range(2):
                    if tMd is None:
                        nc.vector.tensor_scalar(
                            tu[:, 512 * r: 512 * (r + 1)], accs[r][:],
                            0.0, None, AL.add)
                    else:
                        nc.vector.scalar_tensor_tensor(
                            tu[:, 512 * r: 512 * (r + 1)],
                            tMd[:, 512 * r: 512 * (r + 1)], DECAY, accs[r][:],
                            AL.mult, AL.add)
                if has_bias:
                    nc.vector.tensor_scalar(tu[:], tu[:], tb[:], None, AL.add)
                tMd_new = wk.tile([P, NPIX], F32, tag="md")
                nc.vector.scalar_tensor_tensor(
                    tMd_new[:], tu[:], THRESH, tu[:], AL.is_le, AL.mult)
                nc.vector.tensor_scalar(
                    Sb0[nxt][Ch:P, 1:33, 1:33],
                    tu[Ch:P, :].rearrange("c (h w) -> c h w", h=H),
                    THRESH, None, AL.is_gt)
                nc.vector.tensor_scalar(
                    Sb1[nxt][0:Ch, 1:33, 1:33],
                    tu[0:Ch, :].rearrange("c (h w) -> c h w", h=H),
                    THRESH, None, AL.is_gt)
                nc.sync.dma_start(d_y[t, 0], Sb0[nxt][Ch:P, 1:33, 1:33])
                nc.sync.dma_start(d_y[t, 1], Sb1[nxt][0:Ch, 1:33, 1:33])
                tMd = tMd_new
        _pred_span(tc, "f32")
    nc.compile()
    return nc


_cache = {}


def _get_nc(has_bias):
    key = (SCHEME, DROP_E, has_bias)
    if key not in _cache:
        _cache[key] = _build_rs(has_bias) if SCHEME == "rs" else _build_f32(has_bias)
    return _cache[key]


def _pad_frames(a):
    """[..., H, W] -> [..., PW, PWX] zero-padded frames (+dead col)."""
    out = np.zeros(a.shape[:-2] + (PW, PWX), dtype=a.dtype)
    out[..., 1:33, 1:33] = a
    return out


def kernel(data, Wx, bx, Wh, bh, _trace=False):
    data = np.asarray(data, dtype=np.float32)
    Wx = np.asarray(Wx, dtype=np.float32)
    Wh = np.asarray(Wh, dtype=np.float32)
    bx = np.asarray(bx, dtype=np.float32)
    bh = np.asarray(bh, dtype=np.float32)

    # lhsT tap layout: [tap(ky*3+kx), cin, cout]
    WxT = np.ascontiguousarray(np.transpose(Wx, (2, 3, 1, 0)).reshape(9, Cin, Ch))
    WhT = np.ascontiguousarray(np.transpose(Wh, (2, 3, 1, 0)).reshape(9, Ch, Ch))
    bsum = (bx + bh).astype(np.float32)
    has_bias = bool(np.any(bsum))
    bias_v = np.concatenate([bsum, bsum])[:, None]

    nc = _get_nc(has_bias)

    in_maps = []
    if SCHEME == "rs":
        wxh = to_f32r(WxT); wxl = to_f32r(WxT - wxh)
        whh = to_f32r(WhT); whl = to_f32r(WhT - whh)
        W1 = np.concatenate([wxh, wxh], axis=1)          # S1 = [xh; xl]
        W2 = np.concatenate([whh, wxl], axis=1)          # S2 = [h; xh]
        WE = np.concatenate([whl, np.zeros_like(whl)], axis=1)
        for c in range(NCORES):
            xc = np.moveaxis(data[BLOC * c: BLOC * (c + 1)], -1, 0)  # [T,2,64,32,32]
            xh = to_f32r(xc)
            xl = to_f32r(xc - xh)
            xS1 = _pad_frames(np.concatenate([xh, xl], axis=2))
            xS2 = _pad_frames(xh)
            in_maps.append({"xS1": xS1, "xS2": xS2, "W1": W1, "W2": W2,
                            "WE": WE, "bias": bias_v})
    else:
        Wb0 = np.concatenate([WxT, WhT], axis=1)  # b0: [x; h]
        Wb1 = np.concatenate([WhT, WxT], axis=1)  # b1: [h; x]
        for c in range(NCORES):
            xc = _pad_frames(np.moveaxis(data[BLOC * c: BLOC * (c + 1)], -1, 0))
            in_maps.append({"xS1": xc, "W2b0": Wb0, "W2b1": Wb1, "bias": bias_v})

    res = run_bass_kernel_spmd(nc, in_maps, core_ids=list(range(NCORES)),
                               trace=_trace)
    out = np.empty((B, Ch, H, W, T), dtype=np.float32)
    for c in range(NCORES):
        y = res.results[c]["y"]  # [T, 2, 64, 32, 32]
        for j in range(BLOC):
            out[BLOC * c + j] = np.transpose(y[:, j], (1, 2, 3, 0))
    if _trace:
        return out, res
    return out
